# revision 14
# baseline (speedup 1.0000x reference)
"""Trainium2 Bass kernel for nn_InvariantCrossAttention.

Math: the reference computes softmax(-(Q2_i + K2_j), axis=j) — but -Q2_i is
constant along the softmax axis, so it cancels. The attention row is the same
for every query i, hence context[b,i] is i-independent and the final mean over
N is a no-op:

    out[b] = sum_j exp(-K2[b,j]) * K2[b,j] / sum_j exp(-K2[b,j])
    K2[b,j] = (x[b,j] - mean_j x[b,:])^2,  x = all_atom_features[:, :, 0]

cdr3_features does not affect the output (for any input values). Every core
runs the full (replicated) computation — a cross-core split would put a
multi-us collective on a sub-us critical path.

This version is raw Bass (no TileContext): the profiler's measured window
starts at the first BIR-named instruction and ends at the end of the NEFF's
fixed semaphore-reset epilogue, so the Tile preamble (const memsets, barrier,
~1.2us) was pure overhead. Structure:

  - x viewed as [128 part, 256 cols]; partition p holds batch p//32.
  - Input DMA split across the two HWDGE rings (SP + Activation).
  - Per-batch -mean lands per-partition via ONE matmul against a memset-built
    block-diagonal [128,128] bf16 constant (value -1/8192 exactly).
  - exp(-t^2) comes from one Derivative_Erf activation (= 2/sqrt(pi)*e^{-t^2};
    the constant cancels in the ratio), with fused per-partition accumulation
    for sum(w). DVE computes t^2 and w*t^2 (accumulating sum(w*t^2)) in
    parallel with the Scalar engine.
  - Final per-batch sums via one matmul with the accumulator columns as the
    stationary operand, giving [2,4] in PSUM: row 0 = sum(w), row 1 = sum(wk),
    batches along the free dim, so the result lives on one partition and the
    output DMA is a single contiguous 16B packet.
  - No explicit wait on the output DMA: the NEFF epilogue's post-barrier queue
    DRAIN covers it after the (longer) semaphore-reset tail.
"""

import os

import numpy as np

B = 4  # batch
M = 8192  # all_atom length (softmax axis)
P = 128  # SBUF partitions
COLS = B * M // P  # 256 elements per partition
PPB = P // B  # 32 partitions per batch
N_CORES = 8

_cache = {}
last_results = None  # BassKernelResults of the most recent run (for test.py)


def _build():
    import concourse.bacc as bacc
    import concourse.bass as bass
    import concourse.mybir as mybir

    f32 = mybir.dt.float32
    bf16 = mybir.dt.bfloat16
    AF = mybir.ActivationFunctionType
    ALU = mybir.AluOpType
    nc = bacc.Bacc("TRN2", target_bir_lowering=False, debug=False)

    x_dram = nc.dram_tensor("x", [P, COLS], f32, kind="ExternalInput")
    out_dram = nc.dram_tensor("out", [1, B], f32, kind="ExternalOutput")

    H = P // 2
    from contextlib import ExitStack

    with ExitStack() as es:
        X = es.enter_context(nc.sbuf_tensor([P, COLS], f32))
        BO = es.enter_context(nc.sbuf_tensor([P, P], bf16))  # block-diag -1/M
        MK = es.enter_context(nc.sbuf_tensor([P, B], bf16))  # block mask (ones)
        ps = es.enter_context(nc.sbuf_tensor([P, 1], bf16))  # per-part col sums
        nm = es.enter_context(nc.sbuf_tensor([P, 1], f32))  # -mean per partition
        w = es.enter_context(nc.sbuf_tensor([P, COLS], bf16))  # ~exp(-t^2)
        t = es.enter_context(nc.sbuf_tensor([P, COLS], bf16))  # x - mean
        t2 = es.enter_context(nc.sbuf_tensor([P, COLS], bf16))  # t^2
        wk = es.enter_context(nc.sbuf_tensor([P, COLS], bf16))  # w * t^2
        acc = es.enter_context(nc.sbuf_tensor([P, 2], f32))  # [sum w | sum wk]
        accb = es.enter_context(nc.sbuf_tensor([P, 2], bf16))  # bf16 for matmul
        rcp = es.enter_context(nc.sbuf_tensor([1, B], f32))  # 1/sum(w)
        res = es.enter_context(nc.sbuf_tensor([1, B], f32))  # final out
        NM = es.enter_context(nc.psum_tensor([P, 1], f32))
        SW = es.enter_context(nc.psum_tensor([1, B], f32))  # sum(w) per batch
        SK = es.enter_context(nc.psum_tensor([1, B], f32))  # sum(wk) per batch
        dS = es.enter_context(nc.semaphore())  # input DMA completion (+16/half)
        vD = es.enter_context(nc.semaphore())  # DVE same-engine completion chain
        vR = es.enter_context(nc.semaphore())  # reduce done
        tN = es.enter_context(nc.semaphore())  # NM matmul done
        vC = es.enter_context(nc.semaphore())  # nm copy done
        sW = es.enter_context(nc.semaphore())  # DErf (w + acc col0) done
        vK = es.enter_context(nc.semaphore())  # both acc cols cast to bf16
        tS = es.enter_context(nc.semaphore())  # both sum matmuls done
        vF = es.enter_context(nc.semaphore())  # res ready
        dO = es.enter_context(nc.semaphore())  # output DMA completion (unwaited)
        with nc.Block(no_gpsimd_drain=True) as block:

            @block.sync
            def _(sync):
                sync.dma_start(X[0:H, :], x_dram[0:H, :]).then_inc(dS, 16)

            @block.scalar
            def _(scalar):
                scalar.dma_start(X[H:P, :], x_dram[H:P, :]).then_inc(dS, 16)
                scalar.wait_ge(vC, 1)
                # then_inc lands on the lowered ACTIVATION_READ_ACCUMULATOR,
                # so sW also covers acc[:, 0].
                scalar.activation(
                    w[:], X[:], AF.Derivative_Erf, bias=nm[:], accum_out=acc[:, 0:1]
                ).then_inc(sW, 1)

            @block.vector
            def _(vector):
                # Constants, built while the input DMA is in flight. Engines
                # complete out of order, so same-engine RAW/WAW deps are
                # chained through vD (waits fuse into the next instruction).
                # BO row-bands are mutually disjoint: no chaining among them.
                for b in range(B):
                    vector.memset(BO[b * PPB : (b + 1) * PPB, :], 0.0).then_inc(
                        vD, 1
                    )
                for b in range(B):
                    vector.wait_ge(vD, B)
                    vector.memset(
                        BO[b * PPB : (b + 1) * PPB, b * PPB : (b + 1) * PPB],
                        -1.0 / M,
                    ).then_inc(vD, 1)
                vector.memset(MK[:], 0.0).then_inc(vD, 1)
                for b in range(B):
                    vector.wait_ge(vD, 2 * B + 1)
                    vector.memset(
                        MK[b * PPB : (b + 1) * PPB, b : b + 1], 1.0
                    ).then_inc(vD, 1)

                vector.wait_ge(dS, 32)
                vector.wait_ge(vD, 3 * B + 1)  # all memsets retired
                with nc.allow_low_precision(reason="col sums feed a bf16 matmul"):
                    vector.tensor_reduce(
                        ps[:], X[:], axis=mybir.AxisListType.X, op=ALU.add
                    ).then_inc(vR, 1)
                vector.wait_ge(tN, 1)
                vector.tensor_copy(nm[:], NM[:]).then_inc(vC, 1)
                vector.wait_ge(vC, 1)  # own copy retired before reading nm
                vector.tensor_scalar_add(t[:], X[:], nm[:]).then_inc(vD, 1)
                vector.wait_ge(vD, 3 * B + 2)
                vector.tensor_tensor(t2[:], t[:], t[:], op=ALU.mult).then_inc(vD, 1)
                vector.wait_ge(sW, 1)
                vector.wait_ge(vD, 3 * B + 3)
                vector.scalar_tensor_tensor(
                    wk[:], w[:], 1.0, t2[:],
                    op0=ALU.mult, op1=ALU.mult, accum_out=acc[:, 1:2],
                ).then_inc(vD, 1)
                vector.wait_ge(vD, 3 * B + 4)
                vector.tensor_copy(accb[:], acc[:]).then_inc(vK, 1)
                vector.wait_ge(tS, 2)
                vector.reciprocal(rcp[:], SW[:]).then_inc(vD, 1)
                vector.wait_ge(vD, 3 * B + 5)
                vector.tensor_tensor(
                    res[:], SK[:], rcp[:], op=ALU.mult
                ).then_inc(vF, 1)

            @block.tensor
            def _(tensor):
                tensor.wait_ge(vR, 1)
                tensor.matmul(NM[:], BO[:], ps[:]).then_inc(tN, 1)
                tensor.wait_ge(vK, 1)
                tensor.matmul(SW[:], accb[:, 0:1], MK[:]).then_inc(tS, 1)
                tensor.matmul(SK[:], accb[:, 1:2], MK[:]).then_inc(tS, 1)

        # Output DMA issued after the Block so the block-exit drains don't
        # wait out its flight; the NEFF epilogue's post-barrier queue DRAIN
        # guarantees it lands before execution completes. The completion sem
        # is required by the descriptor but never waited on.
        nc.sync.wait_ge(vF, 1)
        nc.sync.dma_start(out_dram[:, :], res[:, :]).then_inc(dO, 16)

    # The Bass constructor unconditionally memsets four never-read const
    # scalars at program start; they are dead code here but would start the
    # profiler's measured window ~1.3us before the input DMA. Drop them.
    for func in nc.m.functions:
        for blk in func.blocks:
            dead = [
                i
                for i in blk.instructions
                if isinstance(i, mybir.InstMemset)
                and i.outs
                and str(getattr(i.outs[0], "memref", "")).startswith("const-")
            ]
            if dead:
                keep = [i for i in blk.instructions if i not in dead]
                blk.instructions = keep

    nc.compile()
    return nc


def kernel(cdr3_features=None, all_atom_features=None, **_unused):
    from concourse.bass_utils import run_bass_kernel_spmd

    global last_results
    if "nc" not in _cache:
        _cache["nc"] = _build()
    nc = _cache["nc"]

    x = np.ascontiguousarray(np.asarray(all_atom_features, dtype=np.float32)).reshape(
        P, COLS
    )
    in_map = {"x": x}

    trace = bool(os.environ.get("KERNEL_TRACE"))
    last_results = run_bass_kernel_spmd(
        nc, [in_map] * N_CORES, list(range(N_CORES)), trace=trace
    )
    out = np.asarray(last_results.results[0]["out"], dtype=np.float32)
    return out.reshape(B, 1)


# revision 16
# speedup vs baseline: 1.2708x; 1.2708x over previous
"""Trainium2 Bass kernel for nn_InvariantCrossAttention.

Math: the reference computes softmax(-(Q2_i + K2_j), axis=j) — but -Q2_i is
constant along the softmax axis, so it cancels. The attention row is the same
for every query i, hence context[b,i] is i-independent and the final mean over
N is a no-op:

    out[b] = sum_j exp(-K2[b,j]) * K2[b,j] / sum_j exp(-K2[b,j])
    K2[b,j] = (x[b,j] - mean_j x[b,:])^2,  x = all_atom_features[:, :, 0]

cdr3_features does not affect the output (for any input values). Every core
runs the full (replicated) computation — a cross-core split would put a
multi-us collective on a sub-us critical path.

This version is raw Bass (no TileContext): the profiler's measured window
starts at the first BIR-named instruction and ends at the end of the NEFF's
fixed semaphore-reset epilogue, so the Tile preamble (const memsets, barrier,
~1.2us) was pure overhead. Structure:

  - x viewed as [128 part, 256 cols]; partition p holds batch p//32.
  - Input DMA split across the two HWDGE rings (SP + Activation).
  - Per-batch -mean lands per-partition via ONE matmul against a memset-built
    block-diagonal [128,128] bf16 constant (value -1/8192 exactly).
  - exp(-t^2) comes from one Derivative_Erf activation (= 2/sqrt(pi)*e^{-t^2};
    the constant cancels in the ratio), with fused per-partition accumulation
    for sum(w). DVE computes t^2 and w*t^2 (accumulating sum(w*t^2)) in
    parallel with the Scalar engine.
  - Final per-batch sums via one matmul with the accumulator columns as the
    stationary operand, giving [2,4] in PSUM: row 0 = sum(w), row 1 = sum(wk),
    batches along the free dim, so the result lives on one partition and the
    output DMA is a single contiguous 16B packet.
  - No explicit wait on the output DMA: the NEFF epilogue's post-barrier queue
    DRAIN covers it after the (longer) semaphore-reset tail.
"""

import os

import numpy as np

B = 4  # batch
M = 8192  # all_atom length (softmax axis)
P = 128  # SBUF partitions
COLS = B * M // P  # 256 elements per partition
PPB = P // B  # 32 partitions per batch
N_CORES = 8

_cache = {}
last_results = None  # BassKernelResults of the most recent run (for test.py)


def _build():
    import concourse.bacc as bacc
    import concourse.bass as bass
    import concourse.mybir as mybir

    f32 = mybir.dt.float32
    bf16 = mybir.dt.bfloat16
    AF = mybir.ActivationFunctionType
    ALU = mybir.AluOpType
    nc = bacc.Bacc("TRN2", target_bir_lowering=False, debug=False)

    x_dram = nc.dram_tensor("x", [P, COLS], f32, kind="ExternalInput")
    out_dram = nc.dram_tensor("out", [1, B], f32, kind="ExternalOutput")

    H = P // 2
    from contextlib import ExitStack

    with ExitStack() as es:
        X = es.enter_context(nc.sbuf_tensor([P, COLS], f32))
        BO = es.enter_context(nc.sbuf_tensor([P, P], bf16))  # block-diag -1/M
        MK = es.enter_context(nc.sbuf_tensor([P, B], bf16))  # block mask (ones)
        ps = es.enter_context(nc.sbuf_tensor([P, 1], bf16))  # per-part col sums
        nm = es.enter_context(nc.sbuf_tensor([P, 1], f32))  # -mean per partition
        w = es.enter_context(nc.sbuf_tensor([P, COLS], bf16))  # ~exp(-t^2)
        t = es.enter_context(nc.sbuf_tensor([P, COLS], bf16))  # x - mean
        t2 = es.enter_context(nc.sbuf_tensor([P, COLS], bf16))  # t^2
        wk = es.enter_context(nc.sbuf_tensor([P, COLS], bf16))  # w * t^2
        acc = es.enter_context(nc.sbuf_tensor([P, 2], f32))  # [sum w | sum wk]
        accb = es.enter_context(nc.sbuf_tensor([P, 2], bf16))  # bf16 for matmul
        rcp = es.enter_context(nc.sbuf_tensor([1, B], f32))  # 1/sum(w)
        res = es.enter_context(nc.sbuf_tensor([1, B], f32))  # final out
        NM = es.enter_context(nc.psum_tensor([P, 1], f32))
        SW = es.enter_context(nc.psum_tensor([1, B], f32))  # sum(w) per batch
        SK = es.enter_context(nc.psum_tensor([1, B], f32))  # sum(wk) per batch
        dS = es.enter_context(nc.semaphore())  # input DMA completion (+16/half)
        vD = es.enter_context(nc.semaphore())  # DVE same-engine completion chain
        vR = es.enter_context(nc.semaphore())  # reduce done
        tN = es.enter_context(nc.semaphore())  # NM matmul done
        vC = es.enter_context(nc.semaphore())  # nm copy done
        sW = es.enter_context(nc.semaphore())  # DErf (w + acc col0) done
        vK = es.enter_context(nc.semaphore())  # both acc cols cast to bf16
        tS = es.enter_context(nc.semaphore())  # both sum matmuls done
        vF = es.enter_context(nc.semaphore())  # res ready
        dO = es.enter_context(nc.semaphore())  # output DMA completion (unwaited)
        with nc.Block(no_gpsimd_drain=True) as block:

            @block.sync
            def _(sync):
                sync.dma_start(X[0:H, :], x_dram[0:H, :]).then_inc(dS, 16)
                sync.wait_ge(vF, 1)
                # completion sem required by the descriptor; never waited on —
                # the NEFF epilogue's queue DRAIN covers output landing.
                sync.dma_start(out_dram[:, :], res[:, :]).then_inc(dO, 16)

            @block.scalar
            def _(scalar):
                scalar.dma_start(X[H:P, :], x_dram[H:P, :]).then_inc(dS, 16)
                scalar.wait_ge(vC, 1)
                # then_inc lands on the lowered ACTIVATION_READ_ACCUMULATOR,
                # so sW also covers acc[:, 0].
                scalar.activation(
                    w[:], X[:], AF.Derivative_Erf, bias=nm[:], accum_out=acc[:, 0:1]
                ).then_inc(sW, 1)

            @block.vector
            def _(vector):
                # Constants, built while the input DMA is in flight. Engines
                # complete out of order, so same-engine RAW/WAW deps are
                # chained through vD (waits fuse into the next instruction).
                # BO row-bands are mutually disjoint: no chaining among them.
                for b in range(B):
                    vector.memset(BO[b * PPB : (b + 1) * PPB, :], 0.0).then_inc(
                        vD, 1
                    )
                for b in range(B):
                    vector.wait_ge(vD, B)
                    vector.memset(
                        BO[b * PPB : (b + 1) * PPB, b * PPB : (b + 1) * PPB],
                        -1.0 / M,
                    ).then_inc(vD, 1)
                vector.memset(MK[:], 0.0).then_inc(vD, 1)
                for b in range(B):
                    vector.wait_ge(vD, 2 * B + 1)
                    vector.memset(
                        MK[b * PPB : (b + 1) * PPB, b : b + 1], 1.0
                    ).then_inc(vD, 1)

                vector.wait_ge(dS, 32)
                vector.wait_ge(vD, 3 * B + 1)  # all memsets retired
                with nc.allow_low_precision(reason="col sums feed a bf16 matmul"):
                    vector.tensor_reduce(
                        ps[:], X[:], axis=mybir.AxisListType.X, op=ALU.add
                    ).then_inc(vR, 1)
                vector.wait_ge(tN, 1)
                vector.tensor_copy(nm[:], NM[:]).then_inc(vC, 1)
                vector.wait_ge(vC, 1)  # own copy retired before reading nm
                vector.tensor_scalar_add(t[:], X[:], nm[:]).then_inc(vD, 1)
                vector.wait_ge(vD, 3 * B + 2)
                vector.tensor_tensor(t2[:], t[:], t[:], op=ALU.mult).then_inc(vD, 1)
                vector.wait_ge(sW, 1)
                vector.wait_ge(vD, 3 * B + 3)
                vector.scalar_tensor_tensor(
                    wk[:], w[:], 1.0, t2[:],
                    op0=ALU.mult, op1=ALU.mult, accum_out=acc[:, 1:2],
                ).then_inc(vD, 1)
                vector.wait_ge(vD, 3 * B + 4)
                vector.tensor_copy(accb[:], acc[:]).then_inc(vK, 1)
                vector.wait_ge(tS, 2)
                vector.reciprocal(rcp[:], SW[:]).then_inc(vD, 1)
                vector.wait_ge(vD, 3 * B + 5)
                vector.tensor_tensor(
                    res[:], SK[:], rcp[:], op=ALU.mult
                ).then_inc(vF, 1)

            @block.tensor
            def _(tensor):
                tensor.wait_ge(vR, 1)
                tensor.matmul(NM[:], BO[:], ps[:]).then_inc(tN, 1)
                tensor.wait_ge(vK, 1)
                tensor.matmul(SW[:], accb[:, 0:1], MK[:]).then_inc(tS, 1)
                tensor.matmul(SK[:], accb[:, 1:2], MK[:]).then_inc(tS, 1)

    # The Bass constructor unconditionally memsets four never-read const
    # scalars at program start; they are dead code here but would start the
    # profiler's measured window ~1.3us before the input DMA. Drop them.
    for func in nc.m.functions:
        for blk in func.blocks:
            dead = [
                i
                for i in blk.instructions
                if isinstance(i, mybir.InstMemset)
                and i.outs
                and str(getattr(i.outs[0], "memref", "")).startswith("const-")
            ]
            if dead:
                keep = [i for i in blk.instructions if i not in dead]
                blk.instructions = keep

    nc.compile()
    return nc


def kernel(cdr3_features=None, all_atom_features=None, **_unused):
    from concourse.bass_utils import run_bass_kernel_spmd

    global last_results
    if "nc" not in _cache:
        _cache["nc"] = _build()
    nc = _cache["nc"]

    x = np.ascontiguousarray(np.asarray(all_atom_features, dtype=np.float32)).reshape(
        P, COLS
    )
    in_map = {"x": x}

    trace = bool(os.environ.get("KERNEL_TRACE"))
    last_results = run_bass_kernel_spmd(
        nc, [in_map] * N_CORES, list(range(N_CORES)), trace=trace
    )
    out = np.asarray(last_results.results[0]["out"], dtype=np.float32)
    return out.reshape(B, 1)


# revision 17
# speedup vs baseline: 1.3166x; 1.0361x over previous
"""Trainium2 Bass kernel for nn_InvariantCrossAttention.

Math: the reference computes softmax(-(Q2_i + K2_j), axis=j) — but -Q2_i is
constant along the softmax axis, so it cancels. The attention row is the same
for every query i, hence context[b,i] is i-independent and the final mean over
N is a no-op:

    out[b] = sum_j exp(-K2[b,j]) * K2[b,j] / sum_j exp(-K2[b,j])
    K2[b,j] = (x[b,j] - mean_j x[b,:])^2,  x = all_atom_features[:, :, 0]

cdr3_features does not affect the output (for any input values). Every core
runs the full (replicated) computation — a cross-core split would put a
multi-us collective on a sub-us critical path.

This version is raw Bass (no TileContext): the profiler's measured window
starts at the first BIR-named instruction and ends at the end of the NEFF's
fixed semaphore-reset epilogue, so the Tile preamble (const memsets, barrier,
~1.2us) was pure overhead. Structure:

  - x viewed as [128 part, 256 cols]; partition p holds batch p//32.
  - Input DMA split across the two HWDGE rings (SP + Activation).
  - Per-batch -mean lands per-partition via ONE matmul against a memset-built
    block-diagonal [128,128] bf16 constant (value -1/8192 exactly).
  - exp(-t^2) comes from one Derivative_Erf activation (= 2/sqrt(pi)*e^{-t^2};
    the constant cancels in the ratio), with fused per-partition accumulation
    for sum(w). DVE computes t^2 and w*t^2 (accumulating sum(w*t^2)) in
    parallel with the Scalar engine.
  - Final per-batch sums via one matmul with the accumulator columns as the
    stationary operand, giving [2,4] in PSUM: row 0 = sum(w), row 1 = sum(wk),
    batches along the free dim, so the result lives on one partition and the
    output DMA is a single contiguous 16B packet.
  - No explicit wait on the output DMA: the NEFF epilogue's post-barrier queue
    DRAIN covers it after the (longer) semaphore-reset tail.
"""

import os

import numpy as np

B = 4  # batch
M = 8192  # all_atom length (softmax axis)
P = 128  # SBUF partitions
COLS = B * M // P  # 256 elements per partition
PPB = P // B  # 32 partitions per batch
N_CORES = 8

_cache = {}
last_results = None  # BassKernelResults of the most recent run (for test.py)


def _build():
    import concourse.bacc as bacc
    import concourse.bass as bass
    import concourse.mybir as mybir

    f32 = mybir.dt.float32
    bf16 = mybir.dt.bfloat16
    AF = mybir.ActivationFunctionType
    ALU = mybir.AluOpType
    nc = bacc.Bacc("TRN2", target_bir_lowering=False, debug=False)

    x_dram = nc.dram_tensor("x", [P, COLS], f32, kind="ExternalInput")
    out_dram = nc.dram_tensor("out", [1, B], f32, kind="ExternalOutput")

    H = P // 2
    from contextlib import ExitStack

    with ExitStack() as es:
        X = es.enter_context(nc.sbuf_tensor([P, COLS], f32))
        BO = es.enter_context(nc.sbuf_tensor([P, P], bf16))  # block-diag -1/M
        MK = es.enter_context(nc.sbuf_tensor([P, B], bf16))  # block mask (ones)
        ps = es.enter_context(nc.sbuf_tensor([P, 1], bf16))  # per-part col sums
        nm = es.enter_context(nc.sbuf_tensor([P, 1], f32))  # -mean per partition
        w = es.enter_context(nc.sbuf_tensor([P, COLS], bf16))  # ~exp(-t^2)
        t = es.enter_context(nc.sbuf_tensor([P, COLS], bf16))  # x - mean
        t2 = es.enter_context(nc.sbuf_tensor([P, COLS], bf16))  # t^2
        wk = es.enter_context(nc.sbuf_tensor([P, COLS], bf16))  # w * t^2
        acc = es.enter_context(nc.sbuf_tensor([P, 2], f32))  # [sum w | sum wk]
        accb = es.enter_context(nc.sbuf_tensor([P, 2], bf16))  # bf16 for matmul
        rcp = es.enter_context(nc.sbuf_tensor([1, B], f32))  # 1/sum(w)
        res = es.enter_context(nc.sbuf_tensor([1, B], f32))  # final out
        NM = es.enter_context(nc.psum_tensor([P, 1], f32))
        SW = es.enter_context(nc.psum_tensor([1, B], f32))  # sum(w) per batch
        SK = es.enter_context(nc.psum_tensor([1, B], f32))  # sum(wk) per batch
        dS = es.enter_context(nc.semaphore())  # input DMA completion (+16/half)
        vD = es.enter_context(nc.semaphore())  # DVE same-engine completion chain
        vR = es.enter_context(nc.semaphore())  # reduce done
        tN = es.enter_context(nc.semaphore())  # NM matmul done
        vC = es.enter_context(nc.semaphore())  # nm copy done
        sW = es.enter_context(nc.semaphore())  # DErf (w + acc col0) done
        vK = es.enter_context(nc.semaphore())  # both acc cols cast to bf16
        tS = es.enter_context(nc.semaphore())  # both sum matmuls done
        vF = es.enter_context(nc.semaphore())  # res ready
        dO = es.enter_context(nc.semaphore())  # output DMA completion (unwaited)
        # Block-free emission: each engine executes its instructions from
        # `main` in program order; cross-engine and same-engine (out-of-order
        # completion) dependencies are all explicit semaphores. No Block exit
        # drains/barrier — engines run straight into the NEFF epilogue's own
        # rendezvous.
        nc.sync.dma_start(X[0:H, :], x_dram[0:H, :]).then_inc(dS, 16)

        nc.scalar.dma_start(X[H:P, :], x_dram[H:P, :]).then_inc(dS, 16)
        nc.scalar.wait_ge(vC, 1)
        # then_inc lands on the lowered ACTIVATION_READ_ACCUMULATOR,
        # so sW also covers acc[:, 0].
        nc.scalar.activation(
            w[:], X[:], AF.Derivative_Erf, bias=nm[:], accum_out=acc[:, 0:1]
        ).then_inc(sW, 1)

        # Constants, built while the input DMA is in flight. BO row-bands are
        # mutually disjoint: no chaining among them.
        for b in range(B):
            nc.vector.memset(BO[b * PPB : (b + 1) * PPB, :], 0.0).then_inc(vD, 1)
        for b in range(B):
            nc.vector.wait_ge(vD, B)
            nc.vector.memset(
                BO[b * PPB : (b + 1) * PPB, b * PPB : (b + 1) * PPB], -1.0 / M
            ).then_inc(vD, 1)
        nc.vector.memset(MK[:], 0.0).then_inc(vD, 1)
        for b in range(B):
            nc.vector.wait_ge(vD, 2 * B + 1)
            nc.vector.memset(MK[b * PPB : (b + 1) * PPB, b : b + 1], 1.0).then_inc(
                vD, 1
            )

        nc.vector.wait_ge(dS, 32)
        nc.vector.wait_ge(vD, 3 * B + 1)  # all memsets retired
        with nc.allow_low_precision(reason="col sums feed a bf16 matmul"):
            nc.vector.tensor_reduce(
                ps[:], X[:], axis=mybir.AxisListType.X, op=ALU.add
            ).then_inc(vR, 1)
        nc.vector.wait_ge(tN, 1)
        nc.vector.tensor_copy(nm[:], NM[:]).then_inc(vC, 1)
        nc.vector.wait_ge(vC, 1)  # own copy retired before reading nm
        nc.vector.tensor_scalar_add(t[:], X[:], nm[:]).then_inc(vD, 1)
        nc.vector.wait_ge(vD, 3 * B + 2)
        nc.vector.tensor_tensor(t2[:], t[:], t[:], op=ALU.mult).then_inc(vD, 1)
        nc.vector.wait_ge(sW, 1)
        nc.vector.wait_ge(vD, 3 * B + 3)
        nc.vector.scalar_tensor_tensor(
            wk[:], w[:], 1.0, t2[:],
            op0=ALU.mult, op1=ALU.mult, accum_out=acc[:, 1:2],
        ).then_inc(vD, 1)
        nc.vector.wait_ge(vD, 3 * B + 4)
        nc.vector.tensor_copy(accb[:], acc[:]).then_inc(vK, 1)
        nc.vector.wait_ge(tS, 2)
        nc.vector.reciprocal(rcp[:], SW[:]).then_inc(vD, 1)
        nc.vector.wait_ge(vD, 3 * B + 5)
        nc.vector.tensor_tensor(res[:], SK[:], rcp[:], op=ALU.mult).then_inc(vF, 1)

        nc.tensor.wait_ge(vR, 1)
        nc.tensor.matmul(NM[:], BO[:], ps[:]).then_inc(tN, 1)
        nc.tensor.wait_ge(vK, 1)
        nc.tensor.matmul(SW[:], accb[:, 0:1], MK[:]).then_inc(tS, 1)
        nc.tensor.matmul(SK[:], accb[:, 1:2], MK[:]).then_inc(tS, 1)

        # Output DMA from the otherwise-idle GpSimd SWDGE queue: its issue
        # cost doesn't delay any critical-path engine's arrival at the NEFF
        # epilogue rendezvous. Completion sem required but never waited on —
        # the epilogue's queue drain covers output landing.
        nc.gpsimd.wait_ge(vF, 1)
        nc.gpsimd.dma_start(out_dram[:, :], res[:, :]).then_inc(dO, 16)

    # The Bass constructor unconditionally memsets four never-read const
    # scalars at program start; they are dead code here but would start the
    # profiler's measured window ~1.3us before the input DMA. Drop them.
    for func in nc.m.functions:
        for blk in func.blocks:
            dead = [
                i
                for i in blk.instructions
                if isinstance(i, mybir.InstMemset)
                and i.outs
                and str(getattr(i.outs[0], "memref", "")).startswith("const-")
            ]
            if dead:
                keep = [i for i in blk.instructions if i not in dead]
                blk.instructions = keep

    nc.compile()
    return nc


def kernel(cdr3_features=None, all_atom_features=None, **_unused):
    from concourse.bass_utils import run_bass_kernel_spmd

    global last_results
    if "nc" not in _cache:
        _cache["nc"] = _build()
    nc = _cache["nc"]

    x = np.ascontiguousarray(np.asarray(all_atom_features, dtype=np.float32)).reshape(
        P, COLS
    )
    in_map = {"x": x}

    trace = bool(os.environ.get("KERNEL_TRACE"))
    last_results = run_bass_kernel_spmd(
        nc, [in_map] * N_CORES, list(range(N_CORES)), trace=trace
    )
    out = np.asarray(last_results.results[0]["out"], dtype=np.float32)
    return out.reshape(B, 1)
